# revision 1
# baseline (speedup 1.0000x reference)
"""MoE routed matmul kernel for Trainium2 (8 NeuronCores, expert-parallel).

Problem: out[b, u] = sum_d x[b, d] * embeddings[content_idx[b], d, u]
with B=256 examples, D=U=1024, C=64 experts (256 MB fp32 table).

Strategy (expert parallel):
  - Core k owns experts [8k, 8k+8). It streams its 8 expert matrices
    (32 MB) from HBM once — the memory roofline for this problem.
  - The host groups examples by expert (pure index bookkeeping), packs
    each group into CAP padded slots, and lays the grouped x out in the
    exact transposed SBUF layout the PE wants (lhsT = x^T per k-chunk).
  - On device, per expert: out[slots, u] = sum_k xT_chunk.T @ W_chunk,
    accumulated in PSUM over 8 k-chunks of 128, with U split in two
    512-wide PSUM banks.
  - Host scatters the padded per-slot outputs back to example order.

The contraction index d is permuted as d = p*8 + b (p = partition,
b = k-chunk) identically on both x and W, which makes every weight DMA
read fully contiguous HBM (the host pre-lays the SBUF image).

Numerics ("bf16x2" variant, default): x and W are each split exactly
into bf16 hi + lo halves (x = xh + xl, W = Wh + Wl, reconstruction
accurate to ~2^-17 relative). The PE accumulates all four cross
products xh@Wh + xl@Wh + xh@Wl + xl@Wl in fp32 PSUM by stacking
[xh; xl] as the stationary operand and streaming Wh and Wl into the
same accumulation group, then a DVE copy+add folds the two slot
halves. This matches fp32 to ~1e-6 rms while running the PE at bf16
rate (fp32 matmuls cost 4 cycles/row on trn2, bf16 one), keeping the
kernel DMA-bound. Each expert's weights stream in four 1 MB chunks so
PE idle gaps stay below the ~3.4 us HAM re-throttle window. The "fp32"
variant is the exact 4-cycle fallback.
"""

import numpy as np
import ml_dtypes

from concourse import bacc, mybir, tile
from concourse import bass_utils

BF16 = ml_dtypes.bfloat16

B, D, U, C = 256, 1024, 1024, 64
NCORES = 8
EPC = C // NCORES          # experts per core
KC = D // 128              # k-chunks per expert
NCH = U // 512             # psum n-chunks per expert

_compiled = {}


def _build_fp32(cap: int):
    """Exact-fp32 per-core SPMD program (PE at 4 cycles/row)."""
    f32 = mybir.dt.float32
    nc = bacc.Bacc("TRN2", target_bir_lowering=False, debug=False)
    w = nc.dram_tensor("w", [EPC, D, U], f32, kind="ExternalInput").ap()
    xt = nc.dram_tensor("xt", [128, EPC * KC * cap], f32, kind="ExternalInput").ap()
    out = nc.dram_tensor("out", [EPC, cap, U], f32, kind="ExternalOutput").ap()

    with tile.TileContext(nc) as tc:
        with tc.tile_pool(name="wp", bufs=2) as wp, \
             tc.tile_pool(name="xp", bufs=1) as xp, \
             tc.tile_pool(name="pp", bufs=4, space="PSUM") as pp, \
             tc.tile_pool(name="op", bufs=3) as op:
            xt_t = xp.tile([128, EPC * KC * cap], f32)
            nc.sync.dma_start(xt_t[:], xt[:])
            for e in range(EPC):
                # whole expert weight as [128, KC*U]; d = p*KC + b, so the
                # HBM read is fully contiguous per partition (32 KB).
                w_t = wp.tile([128, KC * U], f32)
                nc.sync.dma_start(
                    w_t[:].rearrange("p (b u) -> p b u", b=KC),
                    w[e].rearrange("(p b) u -> p b u", b=KC),
                )
                for m0 in range(0, cap, 128):
                    mm = min(128, cap - m0)
                    ps = pp.tile([mm, U], f32)
                    for j in range(NCH):
                        for b in range(KC):
                            fo = e * KC * cap + b * cap + m0
                            nc.tensor.matmul(
                                ps[:, j * 512:(j + 1) * 512],
                                lhsT=xt_t[:, fo:fo + mm],
                                rhs=w_t[:, b * U + j * 512: b * U + j * 512 + 512],
                                start=(b == 0),
                                stop=(b == KC - 1),
                            )
                    ot = op.tile([mm, U], f32)
                    nc.vector.tensor_copy(ot[:], ps[:])
                    nc.sync.dma_start(out[e, m0:m0 + mm, :], ot[:])
    nc.compile()
    return nc


def _build_bf16x2(cap: int):
    """bf16 hi/lo split per-core SPMD program (PE at 1 cycle/row).

    whl holds the host-prepared SBUF image: whl[e, p, (2b+wi)*U + u] =
    W_wi[d = p*KC + b, u] (wi: 0=hi, 1=lo). lhsT layout per (e, b):
    2*cap columns = [xh slots | xl slots]. Each psum n-chunk is one
    accumulation group of 2*KC matmuls; row i collects xh_i@(Wh+Wl),
    row cap+i collects xl_i@(Wh+Wl), and a DVE copy+add folds them.
    """
    f32 = mybir.dt.float32
    bf16 = mybir.dt.bfloat16
    cap2 = 2 * cap
    assert cap2 <= 128 and cap % 32 == 0
    NBP = 4        # DMA chunks per expert (1 MB each)
    BPK = KC // NBP  # k-chunks per DMA chunk
    nc = bacc.Bacc("TRN2", target_bir_lowering=False, debug=False)
    whl = nc.dram_tensor("whl", [EPC, 128, KC * 2 * U], bf16,
                         kind="ExternalInput").ap()
    xt = nc.dram_tensor("xt", [128, EPC * KC * cap2], bf16,
                        kind="ExternalInput").ap()
    out = nc.dram_tensor("out", [EPC, cap, U], f32, kind="ExternalOutput").ap()

    with tile.TileContext(nc) as tc:
        with tc.tile_pool(name="wp", bufs=3 * NBP + 2) as wp, \
             tc.tile_pool(name="xp", bufs=1) as xp, \
             tc.tile_pool(name="pp", bufs=4, space="PSUM") as pp, \
             tc.tile_pool(name="op", bufs=3) as op:
            xt_t = xp.tile([128, EPC * KC * cap2], bf16)
            # xt must land before the first matmul: SWDGE would take ~15us
            # (1KB packets), so split it across both HWDGE rings ahead of
            # the weight stream (~1.5us each)
            half = EPC * KC * cap2 // 2
            nc.sync.dma_start(xt_t[:, :half], xt[:, :half])
            nc.scalar.dma_start(xt_t[:, half:], xt[:, half:])
            held = []
            for e in range(EPC):
                chunks = []
                for bp in range(NBP):
                    wc = wp.tile([128, 2 * BPK * U], bf16, tag="wc")
                    # alternate the two HWDGE rings (SP + ACT) so weight
                    # streams use both hardware queues
                    eng = nc.sync if (e * NBP + bp) % 2 == 0 else nc.scalar
                    eng.dma_start(
                        wc[:],
                        whl[e][:, bp * 2 * BPK * U:(bp + 1) * 2 * BPK * U],
                    )
                    chunks.append(wc)
                ps = pp.tile([cap2, U], f32)
                for bp in range(NBP):
                    wc = chunks[bp]
                    for bl in range(BPK):
                        b = bp * BPK + bl
                        fo = e * KC * cap2 + b * cap2
                        for wi in range(2):
                            for j in range(NCH):
                                nc.tensor.matmul(
                                    ps[:, j * 512:(j + 1) * 512],
                                    lhsT=xt_t[:, fo:fo + cap2],
                                    rhs=wc[:, (2 * bl + wi) * U + j * 512:
                                            (2 * bl + wi) * U + j * 512 + 512],
                                    start=(bp == 0 and bl == 0 and wi == 0),
                                    stop=(bp == NBP - 1 and bl == BPK - 1
                                          and wi == 1),
                                )
                # fold the two slot halves. DVE may read only one PSUM
                # operand per op: copy hi out, then add lo.
                tmp = op.tile([cap, U], f32, tag="tmp")
                ot = op.tile([cap, U], f32, tag="ot")
                nc.vector.tensor_copy(tmp[:], ps[:cap, :])
                nc.vector.tensor_add(ot[:], tmp[:], ps[cap:cap2, :])
                if e < EPC - 2:
                    # mid-stream outputs ride SWDGE so the HWDGE rings
                    # stay clear for the weight stream
                    nc.gpsimd.dma_start(out[e, :, :], ot[:])
                else:
                    # last two experts' outputs go at the end on the
                    # by-then-idle HWDGE rings (SWDGE is ~2us/DMA and
                    # would stretch the tail)
                    held.append((e, ot))
            for (e, ot), eng in zip(held, (nc.sync, nc.scalar)):
                eng.dma_start(out[e, :, :], ot[:])
    nc.compile()
    return nc


def _get_compiled(cap: int, variant: str):
    key = (cap, variant)
    if key not in _compiled:
        if variant == "fp32":
            _compiled[key] = _build_fp32(cap)
        elif variant == "bf16x2":
            _compiled[key] = _build_bf16x2(cap)
        else:
            raise ValueError(variant)
    return _compiled[key]


def _route(content_idx, x, cap):
    """Group examples by expert into padded slots. Returns the packed
    per-expert x [C, cap, D] plus the (expert, slot) of every example."""
    counts = np.bincount(content_idx, minlength=C)
    order = np.argsort(content_idx, kind="stable")
    cs = content_idx[order]
    starts = np.zeros(C, np.int64)
    starts[1:] = np.cumsum(counts)[:-1]
    slot = np.arange(B) - starts[cs]
    xp_ = np.zeros((C, cap, D), np.float32)
    xp_[cs, slot] = x[order]
    return xp_, order, cs, slot


def _to_lhsT(xp_, cap, dtype):
    """[C, cap, D] packed x -> per-core lhsT layout
    [NCORES, 128, EPC*KC*cap] with free index e*KC*cap + b*cap + i and
    the d = p*KC + b permutation (matching the weight layout)."""
    xt = np.asarray(xp_, dtype).reshape(C, cap, 128, KC)  # [c, i, p, b]
    xt = xt.reshape(NCORES, EPC, cap, 128, KC)
    xt = xt.transpose(0, 3, 1, 4, 2)                      # [k, p, e, b, i]
    return np.ascontiguousarray(xt).reshape(NCORES, 128, EPC * KC * cap)


def run(content_idx, x, embeddings, trace=False, trace_cores=None,
        variant="bf16x2"):
    content_idx = np.asarray(content_idx, np.int32)
    x = np.ascontiguousarray(np.asarray(x, np.float32))
    embeddings = np.ascontiguousarray(np.asarray(embeddings, np.float32))

    counts = np.bincount(content_idx, minlength=C)
    cap = max(16, -(-int(counts.max()) // 16) * 16)
    if variant == "bf16x2":
        # DVE partition access is 32-granular (the lo half starts at
        # partition cap) and stacked [xh; xl] needs 2*cap <= 128.
        cap = max(32, -(-int(counts.max()) // 32) * 32)
        if cap > 64:
            variant = "fp32"
            cap = max(16, -(-int(counts.max()) // 16) * 16)
    xp_, order, cs, slot = _route(content_idx, x, cap)

    nc = _get_compiled(cap, variant)
    if variant == "fp32":
        xt = _to_lhsT(xp_, cap, np.float32)
        in_maps = [
            {"w": embeddings[k * EPC:(k + 1) * EPC], "xt": xt[k]}
            for k in range(NCORES)
        ]
    else:
        w_hi = embeddings.astype(BF16)
        w_lo = (embeddings - w_hi.astype(np.float32)).astype(BF16)
        # SBUF image: [c, p, b, wi, u] contiguous; d = p*KC + b
        whl = np.stack(
            [w_hi.reshape(C, 128, KC, U), w_lo.reshape(C, 128, KC, U)],
            axis=3,
        ).reshape(C, 128, KC * 2 * U)
        x_hi = xp_.astype(BF16)
        x_lo = (xp_ - x_hi.astype(np.float32)).astype(BF16)
        xhl = np.concatenate([x_hi, x_lo], axis=1)  # [C, 2*cap, D]
        xt = _to_lhsT(xhl, 2 * cap, BF16)
        in_maps = [
            {"whl": whl[k * EPC:(k + 1) * EPC], "xt": xt[k]}
            for k in range(NCORES)
        ]

    res = bass_utils.run_bass_kernel_spmd(
        nc, in_maps, core_ids=list(range(NCORES)),
        trace=trace, trace_cores=trace_cores,
    )
    outs = np.stack([res.results[k]["out"] for k in range(NCORES)])
    outs = outs.reshape(C, cap, U)
    out = np.empty((B, U), np.float32)
    out[order] = outs[cs, slot]
    return out, res


def kernel(content_idx, x, embeddings):
    out, _ = run(content_idx, x, embeddings)
    return out



# revision 2
# speedup vs baseline: 2.2406x; 2.2406x over previous
"""MoE routed matmul kernel for Trainium2 (8 NeuronCores, expert-parallel).

Problem: out[b, u] = sum_d x[b, d] * embeddings[content_idx[b], d, u]
with B=256 examples, D=U=1024, C=64 experts (256 MB fp32 table).

Strategy (expert parallel, fp8 weights):
  - Core k owns experts [8k, 8k+8). It streams its 8 expert matrices
    from HBM once. Weights ride as fp8 e3m4 (1 byte/elem) with
    per-column scales applied on the host after gather — 8.4 MB/core,
    the memory roofline, vs 33.5 MB for the bf16 hi/lo baseline.
  - Orientation is flipped vs the baseline: W is the STATIONARY
    operand in [128, 128] blocks (full-width LDWEIGHTS → the backend
    enables FWL, 4 fp8 cols/cycle) and the grouped x is the MOVING
    operand. Per expert that is 64 LDWEIGHTS + 64 short matmuls
    (2*cap rows each) ≈ 1.7 us of PE — PE stays far below the DMA
    stream, so the kernel is DMA-bound at the fp8 roofline.
  - x rides as fp8 hi/lo (x*g = qxh + qxl/32, each e3m4) stacked
    along the moving free dim; a 2-op DVE fold (hi + lo/32) runs per
    expert, overlapped with the stream. e3m4 products are exact in
    the PE datapath, so device numerics match the host simulation;
    end-to-end rel err ~1.3e-2 is set by the fp8 weight rounding.
  - Host: groups examples by expert into cap padded slots, lays W/x
    out in the exact SBUF images the PE wants, and after gather
    applies the per-column weight scales and 1/g (host pre/post is
    index bookkeeping + O(B*U) scaling; all O(B*D*U) math is on
    device).
"""

import numpy as np
import ml_dtypes

from concourse import bacc, mybir, tile
from concourse import bass_utils

E3M4 = ml_dtypes.float8_e3m4

B, D, U, C = 256, 1024, 1024, 64
NCORES = 8
EPC = C // NCORES          # experts per core
KC = D // 128              # 128-deep k-chunks per expert
NJ = U // 128              # 128-wide u-blocks per expert
NBP = 4                    # DMA chunks per expert (2 u-blocks each)

_compiled = {}


def _build_fp8(cap: int):
    """Per-core SPMD program: fp8 e3m4 weights stationary, x moving."""
    f32 = mybir.dt.float32
    fp8 = mybir.dt.float8e3
    cap2 = 2 * cap
    nc = bacc.Bacc("TRN2", target_bir_lowering=False, debug=False)
    whl = nc.dram_tensor("whl", [EPC, 128, KC * NJ * 128], fp8,
                         kind="ExternalInput").ap()
    xt = nc.dram_tensor("xt", [128, EPC * KC * cap2], fp8,
                        kind="ExternalInput").ap()
    out = nc.dram_tensor("out", [EPC, 128, NJ * cap], f32,
                         kind="ExternalOutput").ap()

    psum_bufs = 2 if cap <= 128 else 1
    with tile.TileContext(nc) as tc:
        with tc.tile_pool(name="wp", bufs=10) as wp, \
             tc.tile_pool(name="xp", bufs=1) as xp, \
             tc.tile_pool(name="pp", bufs=psum_bufs, space="PSUM") as pp, \
             tc.tile_pool(name="tp", bufs=2) as tp, \
             tc.tile_pool(name="op", bufs=3) as op:
            xt_t = xp.tile([128, EPC * KC * cap2], fp8)
            nc.sync.dma_start(xt_t[:], xt[:])
            held = []
            for e in range(EPC):
                # whole expert = [128, 8 KB] fp8, contiguous per
                # partition; 4 chunks of 2 KB lines across both HWDGE
                # rings so PE idle gaps stay inside the HAM window.
                chunks = []
                for bp in range(NBP):
                    wc = wp.tile([128, KC * NJ * 128 // NBP], fp8, tag="wc")
                    eng = nc.sync if (e * NBP + bp) % 2 == 0 else nc.scalar
                    eng.dma_start(
                        wc[:],
                        whl[e][:, bp * 2048:(bp + 1) * 2048],
                    )
                    chunks.append(wc)
                ps = pp.tile([128, NJ * cap2], f32)
                for j in range(NJ):
                    wc = chunks[j // 2]
                    for k in range(KC):
                        nc.tensor.matmul(
                            ps[:, j * cap2:(j + 1) * cap2],
                            lhsT=wc[:, ((j % 2) * KC + k) * 128:
                                    ((j % 2) * KC + k) * 128 + 128],
                            rhs=xt_t[:, (e * KC + k) * cap2:
                                     (e * KC + k) * cap2 + cap2],
                            start=(k == 0),
                            stop=(k == KC - 1),
                        )
                # fold the hi/lo x halves: out = hi + lo/32. DVE may
                # read only one PSUM operand per op: scale lo out to
                # SBUF, then add hi.
                tmp = tp.tile([128, NJ * cap], f32, tag="tmp")
                ot = op.tile([128, NJ * cap], f32, tag="ot")
                ps3 = ps[:].rearrange("p (j s) -> p j s", j=NJ)
                tmp3 = tmp[:].rearrange("p (j s) -> p j s", j=NJ)
                ot3 = ot[:].rearrange("p (j s) -> p j s", j=NJ)
                nc.vector.tensor_scalar_mul(tmp3, ps3[:, :, cap:cap2],
                                            1.0 / 32)
                nc.vector.tensor_add(ot3, tmp3, ps3[:, :, 0:cap])
                if e < EPC - 2:
                    # mid-stream outputs ride SWDGE so the HWDGE rings
                    # stay clear for the weight stream
                    nc.gpsimd.dma_start(out[e], ot[:])
                else:
                    held.append((e, ot))
            for (e, ot), eng in zip(held, (nc.sync, nc.scalar)):
                eng.dma_start(out[e], ot[:])
    nc.compile()
    return nc


def _get_compiled(cap: int):
    if cap not in _compiled:
        _compiled[cap] = _build_fp8(cap)
    return _compiled[cap]


def _route(content_idx, x, cap):
    """Group examples by expert into padded slots. Returns the packed
    per-expert x [C, cap, D] plus the (expert, slot) of every example."""
    counts = np.bincount(content_idx, minlength=C)
    order = np.argsort(content_idx, kind="stable")
    cs = content_idx[order]
    starts = np.zeros(C, np.int64)
    starts[1:] = np.cumsum(counts)[:-1]
    slot = np.arange(len(content_idx)) - starts[cs]
    xp_ = np.zeros((C, cap, D), np.float32)
    xp_[cs, slot] = x[order]
    return xp_, order, cs, slot


def run(content_idx, x, embeddings, trace=False, trace_cores=None,
        variant="fp8"):
    content_idx = np.asarray(content_idx, np.int32)
    x = np.ascontiguousarray(np.asarray(x, np.float32))
    embeddings = np.ascontiguousarray(np.asarray(embeddings, np.float32))

    counts = np.bincount(content_idx, minlength=C)
    cap = max(16, -(-int(counts.max()) // 16) * 16)
    xp_, order, cs, slot = _route(content_idx, x, cap)
    cap2 = 2 * cap

    # --- weights: fp8 e3m4 with per-column scales (applied on host) ---
    s_col = np.abs(embeddings).max(axis=1) / 8.0          # [C, U]
    s_col = np.maximum(s_col, 1e-30).astype(np.float32)
    q = (embeddings / s_col[:, None, :]).astype(E3M4)
    # SBUF image: whl[c, p, (j*KC + k)*128 + uu] = q[c, d=k*128+p, u=j*128+uu]
    whl = np.ascontiguousarray(
        q.reshape(C, KC, 128, NJ, 128).transpose(0, 2, 3, 1, 4)
    ).reshape(C, 128, KC * NJ * 128)

    # --- x: fp8 hi/lo, x*g = qxh + qxl/32 ---
    g = np.float32(8.0) / max(float(np.abs(xp_).max()), 1e-30)
    xg = xp_ * g
    xh = xg.astype(E3M4)
    xl = ((xg - xh.astype(np.float32)) * 32.0).astype(E3M4)
    xhl = np.concatenate([xh, xl], axis=1)                # [C, cap2, D]
    # lhsT-side moving image: xt[core, p, (e*KC + k)*cap2 + s]
    xt = np.ascontiguousarray(
        xhl.reshape(NCORES, EPC, cap2, KC, 128).transpose(0, 4, 1, 3, 2)
    ).reshape(NCORES, 128, EPC * KC * cap2)

    nc = _get_compiled(cap)
    in_maps = [
        {"whl": whl[k * EPC:(k + 1) * EPC], "xt": xt[k]}
        for k in range(NCORES)
    ]
    res = bass_utils.run_bass_kernel_spmd(
        nc, in_maps, core_ids=list(range(NCORES)),
        trace=trace, trace_cores=trace_cores,
    )
    # out_dev[core, e, p, j*cap + s] = acc[u = j*128 + p, slot s]
    od = np.stack([res.results[k]["out"] for k in range(NCORES)])
    outs = od.reshape(NCORES, EPC, 128, NJ, cap).transpose(0, 1, 4, 3, 2)
    outs = np.ascontiguousarray(outs).reshape(C, cap, U)
    out = np.empty((len(content_idx), U), np.float32)
    out[order] = outs[cs, slot] * s_col[cs] / g
    return out, res


def kernel(content_idx, x, embeddings):
    out, _ = run(content_idx, x, embeddings)
    return out


# revision 5
# speedup vs baseline: 2.4734x; 1.1039x over previous
"""MoE routed matmul kernel for Trainium2 (8 NeuronCores, expert-parallel).

Problem: out[b, u] = sum_d x[b, d] * embeddings[content_idx[b], d, u]
with B=256 examples, D=U=1024, C=64 experts (256 MB fp32 table).

Strategy (expert parallel, fp8 weights):
  - Core k owns experts [8k, 8k+8). It streams its 8 expert matrices
    from HBM once. Weights ride as fp8 e3m4 (1 byte/elem) with
    per-column scales applied on the host after gather — 8.4 MB/core,
    the memory roofline, vs 33.5 MB for the bf16 hi/lo baseline.
  - Orientation is flipped vs the baseline: W is the STATIONARY
    operand in [128, 128] blocks (full-width LDWEIGHTS → the backend
    enables FWL, 4 fp8 cols/cycle) and the grouped x is the MOVING
    operand. Per expert that is 64 LDWEIGHTS + 64 short matmuls
    (2*cap rows each) ≈ 1.7 us of PE — PE stays far below the DMA
    stream, so the kernel is DMA-bound at the fp8 roofline.
  - x rides as fp8 hi/lo (x*g = qxh + qxl/32, each e3m4) stacked
    along the moving free dim; a 2-op DVE fold (hi + lo/32) runs per
    expert, overlapped with the stream. e3m4 products are exact in
    the PE datapath, so device numerics match the host simulation;
    end-to-end rel err ~1.3e-2 is set by the fp8 weight rounding.
  - Host: groups examples by expert into cap padded slots, lays W/x
    out in the exact SBUF images the PE wants, and after gather
    applies the per-column weight scales and 1/g (host pre/post is
    index bookkeeping + O(B*U) scaling; all O(B*D*U) math is on
    device).
"""

import numpy as np
import ml_dtypes

from concourse import bacc, mybir, tile
from concourse import bass_utils

E3M4 = ml_dtypes.float8_e3m4

B, D, U, C = 256, 1024, 1024, 64
NCORES = 8
EPC = C // NCORES          # experts per core
KC = D // 128              # 128-deep k-chunks per expert
NJ = U // 128              # 128-wide u-blocks per expert
NBP = 2                    # DMA chunks per expert (4 u-blocks each)

_compiled = {}


def _build_fp8(cap: int):
    """Per-core SPMD program: fp8 e3m4 weights stationary, x moving."""
    f32 = mybir.dt.float32
    fp8 = mybir.dt.float8e3
    cap2 = 2 * cap
    nc = bacc.Bacc("TRN2", target_bir_lowering=False, debug=False)
    whl = nc.dram_tensor("whl", [EPC, 128, KC * NJ * 128], fp8,
                         kind="ExternalInput").ap()
    xt = nc.dram_tensor("xt", [128, EPC * KC * cap2], fp8,
                        kind="ExternalInput").ap()
    out = nc.dram_tensor("out", [EPC, 128, NJ * cap], f32,
                         kind="ExternalOutput").ap()

    # per-partition PSUM is 16 KB; a tile is NJ*cap2 fp32 bytes
    psum_bufs = max(1, min(4, 16384 // (NJ * cap2 * 4)))
    with tile.TileContext(nc) as tc:
        with tc.tile_pool(name="wp", bufs=2 * EPC) as wp, \
             tc.tile_pool(name="xp", bufs=1) as xp, \
             tc.tile_pool(name="pp", bufs=psum_bufs, space="PSUM") as pp, \
             tc.tile_pool(name="tp", bufs=3) as tp, \
             tc.tile_pool(name="op", bufs=3) as op:
            xt_t = xp.tile([128, EPC * KC * cap2], fp8)
            nc.sync.dma_start(xt_t[:], xt[:])
            held = []
            for e in range(EPC):
                # whole expert = [128, 8 KB] fp8, contiguous per
                # partition; 2 chunks of 4 KB lines across both HWDGE
                # rings. The wp pool holds the full 8-expert stream so
                # the DMA rings free-run ahead of the PE.
                chunks = []
                jpb = NJ // NBP  # u-blocks per chunk
                for bp in range(NBP):
                    wc = wp.tile([128, KC * NJ * 128 // NBP], fp8, tag="wc")
                    eng = nc.sync if (e * NBP + bp) % 2 == 0 else nc.scalar
                    eng.dma_start(
                        wc[:],
                        whl[e][:, bp * 4096:(bp + 1) * 4096],
                    )
                    chunks.append(wc)
                ps = pp.tile([128, NJ * cap2], f32)
                for j in range(NJ):
                    wc = chunks[j // jpb]
                    for k in range(KC):
                        nc.tensor.matmul(
                            ps[:, j * cap2:(j + 1) * cap2],
                            lhsT=wc[:, ((j % jpb) * KC + k) * 128:
                                    ((j % jpb) * KC + k) * 128 + 128],
                            rhs=xt_t[:, (e * KC + k) * cap2:
                                     (e * KC + k) * cap2 + cap2],
                            start=(k == 0),
                            stop=(k == KC - 1),
                        )
                # fold the hi/lo x halves: out = hi + lo/32. DVE may
                # read only one PSUM operand per op: scale lo out to
                # SBUF, then add hi.
                tmp = tp.tile([128, NJ * cap], f32, tag="tmp")
                ot = op.tile([128, NJ * cap], f32, tag="ot")
                ps3 = ps[:].rearrange("p (j s) -> p j s", j=NJ)
                tmp3 = tmp[:].rearrange("p (j s) -> p j s", j=NJ)
                ot3 = ot[:].rearrange("p (j s) -> p j s", j=NJ)
                nc.vector.tensor_scalar_mul(tmp3, ps3[:, :, cap:cap2],
                                            1.0 / 32)
                nc.vector.tensor_add(ot3, tmp3, ps3[:, :, 0:cap])
                if e < EPC - 2:
                    # mid-stream outputs ride SWDGE so the HWDGE rings
                    # stay clear for the weight stream
                    nc.gpsimd.dma_start(out[e], ot[:])
                else:
                    held.append((e, ot))
            for (e, ot), eng in zip(held, (nc.sync, nc.scalar)):
                eng.dma_start(out[e], ot[:])
    nc.compile()
    return nc


def _get_compiled(cap: int):
    if cap not in _compiled:
        _compiled[cap] = _build_fp8(cap)
    return _compiled[cap]


def _route(content_idx, x, cap):
    """Group examples by expert into padded slots. Returns the packed
    per-expert x [C, cap, D] plus the (expert, slot) of every example."""
    counts = np.bincount(content_idx, minlength=C)
    order = np.argsort(content_idx, kind="stable")
    cs = content_idx[order]
    starts = np.zeros(C, np.int64)
    starts[1:] = np.cumsum(counts)[:-1]
    slot = np.arange(len(content_idx)) - starts[cs]
    xp_ = np.zeros((C, cap, D), np.float32)
    xp_[cs, slot] = x[order]
    return xp_, order, cs, slot


def run(content_idx, x, embeddings, trace=False, trace_cores=None,
        variant="fp8"):
    content_idx = np.asarray(content_idx, np.int32)
    x = np.ascontiguousarray(np.asarray(x, np.float32))
    embeddings = np.ascontiguousarray(np.asarray(embeddings, np.float32))

    counts = np.bincount(content_idx, minlength=C)
    cap = max(16, -(-int(counts.max()) // 16) * 16)
    xp_, order, cs, slot = _route(content_idx, x, cap)
    cap2 = 2 * cap

    # --- weights: fp8 e3m4 with per-column scales (applied on host) ---
    s_col = np.abs(embeddings).max(axis=1) / 8.0          # [C, U]
    s_col = np.maximum(s_col, 1e-30).astype(np.float32)
    q = (embeddings / s_col[:, None, :]).astype(E3M4)
    # SBUF image: whl[c, p, (j*KC + k)*128 + uu] = q[c, d=k*128+p, u=j*128+uu]
    whl = np.ascontiguousarray(
        q.reshape(C, KC, 128, NJ, 128).transpose(0, 2, 3, 1, 4)
    ).reshape(C, 128, KC * NJ * 128)

    # --- x: fp8 hi/lo, x*g = qxh + qxl/32 ---
    g = np.float32(8.0) / max(float(np.abs(xp_).max()), 1e-30)
    xg = xp_ * g
    xh = xg.astype(E3M4)
    xl = ((xg - xh.astype(np.float32)) * 32.0).astype(E3M4)
    xhl = np.concatenate([xh, xl], axis=1)                # [C, cap2, D]
    # lhsT-side moving image: xt[core, p, (e*KC + k)*cap2 + s]
    xt = np.ascontiguousarray(
        xhl.reshape(NCORES, EPC, cap2, KC, 128).transpose(0, 4, 1, 3, 2)
    ).reshape(NCORES, 128, EPC * KC * cap2)

    nc = _get_compiled(cap)
    in_maps = [
        {"whl": whl[k * EPC:(k + 1) * EPC], "xt": xt[k]}
        for k in range(NCORES)
    ]
    res = bass_utils.run_bass_kernel_spmd(
        nc, in_maps, core_ids=list(range(NCORES)),
        trace=trace, trace_cores=trace_cores,
    )
    # out_dev[core, e, p, j*cap + s] = acc[u = j*128 + p, slot s]
    od = np.stack([res.results[k]["out"] for k in range(NCORES)])
    outs = od.reshape(NCORES, EPC, 128, NJ, cap).transpose(0, 1, 4, 3, 2)
    outs = np.ascontiguousarray(outs).reshape(C, cap, U)
    out = np.empty((len(content_idx), U), np.float32)
    out[order] = outs[cs, slot] * s_col[cs] / g
    return out, res


def kernel(content_idx, x, embeddings):
    out, _ = run(content_idx, x, embeddings)
    return out
